# revision 35
# baseline (speedup 1.0000x reference)
"""Trainium2 Bass kernel for ComplexGatingNetwork MoE routing.

Computes, for x = x_real + i*x_imag with x in [B=4, S=2048, D=2048]:
    amp   = |x|            (hypot)
    phase = angle(x)       (atan2)
    scores = [amp, phase] @ W + b          -> [B, S, 64]
    top2 of softmax(scores), renormalized  -> probs [B,S,2], idx [B,S,2]

Math used on device (per element):
    s   = xr^2 + xi^2                       (fused custom DVE op)
    r   = sqrt(s + 1e-30)                   (ACT Sqrt)
    w   = 1/xr                              (ACT Reciprocal)
    qa  = clamp(xi * |w|, +-1e8)            (fused custom DVE op)
    at  = arctan(qa)                        (ACT Arctan)
    th  = xr >= 0 ? at : pi*sgn(at) - at    (fused custom DVE op)
    renormalized top-2 softmax == sigmoid(s_top1 - s_top2)

th == atan2(xi, xr) exactly: for xr >= 0, at = atan(xi/|xr|) = atan2; for
xr < 0, atan2 = sgn(xi)*pi - atan(|xi/xr|)*sgn(xi) = pi*sgn(at) - at since
sgn(at) = sgn(xi).  No cancellation anywhere (the denominator is xr itself).

Scheduling: the three ACT functions (Reciprocal / Sqrt / Arctan) live in
three different ACT table sets (~1.3us per set switch), so the kernel is
phased to load each set exactly once:
  window 1: DMA all xr units; ACT runs all reciprocals  [reciprocal set]
  window 2: DMA all xi units; DVE runs sqsum + qa, ACT streams sqrt,
            PE streams the amp matmuls                  [sqrt set]
  tail:     ACT runs arctan, DVE the quadrant fix, PE the phase matmuls,
            then the top-2 fixup (sigmoid shares arctan's set)  [sigmoid set]

Sharding: token-parallel. Host transposes x to [D, tokens] and repacks
unit-major so each per-unit DMA moves 8KB contiguous per partition; 8192
tokens are sharded across the 8 NeuronCores (1024 tokens each).  The router
weight is replicated.
"""

import math

import numpy as np

import concourse.bass as bass
import concourse.bacc as bacc
import concourse.mybir as mybir
from concourse.tile import TileContext
from concourse import bass_utils

AF = mybir.ActivationFunctionType
ALU = mybir.AluOpType
F32 = mybir.dt.float32
I32 = mybir.dt.int32
U32 = mybir.dt.uint32

B, S, D = 4, 2048, 2048
E = 64
TOPK = 2
N_CORES = 8
TOKENS = B * S                   # 8192
TPC = TOKENS // N_CORES          # tokens per core: 1024
KT = D // 128                    # contraction k-tiles: 16
NPAIR = KT // 2                  # k-pair units: 8
HALF = 512                       # tokens per PSUM scores bank
PI = float(np.float32(math.pi))


def _register_custom(name, spec):
    from concourse import dve_ops
    from concourse.dve_spec import lower, _has_src1
    from concourse.dve_uop import DveOpSpec

    for op in dve_ops.OPS:
        if op.name == name:
            return op
    shas = {}
    for ver in ("v3", "v4"):
        uops = lower(spec, ver=ver)
        shas[ver] = DveOpSpec(name=name, opcode=0, uops=uops,
                              rd1_en=_has_src1(spec)).sha(ver)
    op = dve_ops.DveOp(name, spec, subdim=False, uops_sha=shas)
    dve_ops.OPS.append(op)
    dve_ops.CUSTOM_DVE_SPECS[name] = spec
    dve_ops._SUB_OPCODE_FOR_NAME[name] = (
        dve_ops._CUSTOM_DVE_ROW_BASE + len(dve_ops.OPS) - 1)
    return op


def _make_ops():
    from concourse.dve_spec import Spec, Src0, Src1, Zero, C0, C1, sq, \
        select, maxx, minn

    sqsum = _register_custom(
        "SQSUM_ANT_KERNEL",
        Spec(body=sq(Src0) + sq(Src1),
             reference=lambda in0, in1: in0 * in0 + in1 * in1))
    # qa = clamp(xi * |w|, +-C0)   (Src0 = xi, Src1 = w)
    qabs = _register_custom(
        "QABS_ANT_KERNEL",
        Spec(body=maxx(minn(Src0 * select(Src1 >= Zero, Src1, Zero - Src1),
                            C0), Zero - C0),
             reference=lambda in0, in1, s0: np.clip(
                 in0 * np.abs(in1), -s0, s0).astype(np.float32)))
    # th = at            if xr >= 0
    #      pi*sgn(at)-at if xr < 0      (Src0 = at, Src1 = xr, C0 = pi)
    thfix = _register_custom(
        "THFIX_ANT_KERNEL",
        Spec(body=select(Src1 >= Zero, Src0,
                         select(Src0 >= Zero, C0 - Src0,
                                (Zero - C0) - Src0)),
             reference=lambda in0, in1, s0: np.where(
                 in1 >= 0, in0,
                 np.where(in0 >= 0, s0 - in0, -s0 - in0)).astype(np.float32)))
    return sqsum, qabs, thfix


SQSUM, QABS, THFIX = _make_ops()


def _act_recip(nc, out, in_):
    ins = [nc.scalar.lower_ap(in_),
           mybir.ImmediateValue(dtype=mybir.dt.float32, value=0.0),
           mybir.ImmediateValue(dtype=mybir.dt.float32, value=1.0),
           mybir.ImmediateValue(dtype=mybir.dt.float32, value=0.0)]
    return nc.scalar.add_instruction(
        mybir.InstActivation(
            name=nc.get_next_instruction_name(),
            func=AF.Reciprocal, ins=ins,
            outs=[nc.scalar.lower_ap(out)]))


def _build():
    nc = bacc.Bacc(None, target_bir_lowering=False, debug=False)

    # unit-major, partition-contiguous: [u][p][j*1024 + t]
    xrU = nc.dram_tensor("xrU", [NPAIR, 128, 2 * TPC], F32,
                         kind="ExternalInput")
    xiU = nc.dram_tensor("xiU", [NPAIR, 128, 2 * TPC], F32,
                         kind="ExternalInput")
    wa = nc.dram_tensor("wa", [128, KT * E], F32, kind="ExternalInput")
    wp = nc.dram_tensor("wp", [128, KT * E], F32, kind="ExternalInput")
    bvec = nc.dram_tensor("bvec", [E, 1], F32, kind="ExternalInput")
    ident = nc.dram_tensor("ident", [E, E], F32, kind="ExternalInput")

    NTT = TPC // 128             # token tiles per core: 8
    o_probs = nc.dram_tensor("o_probs", [128, NTT * 2], F32,
                             kind="ExternalOutput")
    o_idx = nc.dram_tensor("o_idx", [128, NTT * 2], I32,
                           kind="ExternalOutput")

    xr_v = xrU.ap()
    xi_v = xiU.ap()

    with TileContext(nc) as tc:
        with tc.tile_pool(name="wpool", bufs=1) as wpool, \
             tc.tile_pool(name="xrp", bufs=1) as xrp, \
             tc.tile_pool(name="xip", bufs=3) as xip, \
             tc.tile_pool(name="wrp", bufs=1) as wrp, \
             tc.tile_pool(name="srp", bufs=2) as srp, \
             tc.tile_pool(name="small", bufs=1) as small, \
             tc.tile_pool(name="pscore", bufs=2, space="PSUM") as pscore, \
             tc.tile_pool(name="ptrans", bufs=2, space="PSUM") as ptrans:

            # ---- window 1 ----
            # sync queue carries only the xr stream (8 back-to-back 1MB
            # transfers) + the first xi units; weights and the gated
            # xi4/xi6 ride the gpsimd SWDGE queue (gates stall only that
            # idle queue); xi3/5/7 issue from the scalar queue inside the
            # window-2 loop, where their buffer-free sems resolve with
            # slack.
            wa_sb = wpool.tile([128, KT, E], F32)
            nc.gpsimd.dma_start(wa_sb[:], wa.ap().rearrange(
                "p (k e) -> p k e", k=KT))
            wp_sb = wpool.tile([128, KT, E], F32)
            nc.gpsimd.dma_start(wp_sb[:], wp.ap().rearrange(
                "p (k e) -> p k e", k=KT))
            b_sb = wpool.tile([E, 1], F32)
            nc.gpsimd.dma_start(b_sb[:], bvec.ap())
            id_sb = wpool.tile([E, E], F32)
            nc.gpsimd.dma_start(id_sb[:], ident.ap())

            # xr units alternate between the two HWDGE queues so the
            # per-instruction issue latency of one queue hides behind the
            # other queue's in-flight transfer.
            xr_tiles = []
            for u in range(NPAIR):
                t = xrp.tile([128, 2, TPC], F32, tag=f"xr{u}")
                eng = nc.sync if u % 2 == 0 else nc.scalar
                eng.dma_start(t[:], xr_v[u].rearrange(
                    "p (j t) -> p j t", j=2))
                xr_tiles.append(t)
            xi_tiles = {}
            xi_issued = set()
            for u in range(NPAIR):
                xi_tiles[u] = xip.tile([128, 2, TPC], F32, tag="xi",
                                       bufs=3, name=f"xi{u}")
            for u in (0, 1, 2, 4, 6):
                nc.sync.dma_start(xi_tiles[u][:], xi_v[u].rearrange(
                    "p (j t) -> p j t", j=2))
                xi_issued.add(u)
            sqrt_bias = wpool.tile([128, 1], F32)
            nc.vector.memset(sqrt_bias[:], 1e-30)

            w_tiles = {}
            recip_insts = []
            for u in range(NPAIR):
                w_u = wrp.tile([128, 2, TPC], F32, tag=f"w{u}")
                recip_insts.append(_act_recip(nc, w_u[:], xr_tiles[u][:]))
                w_tiles[u] = w_u

            ps = [pscore.tile([128, HALF], F32, name=f"ps{h}") for h in range(2)]

            # ---- window 2: xi DMA + sqsum/qa (DVE) + sqrt (ACT) + amp mm ----
            # Score matmuls are column-tiled: even k-tiles accumulate into
            # PSUM partitions 0-63, odd k-tiles into 64-127 (concurrent in
            # the PE array); the two halves are summed after accumulation.
            sqrt_insts = []
            q_tiles = {}
            for u in range(NPAIR):
                xi_u = xi_tiles[u]
                if u not in xi_issued:
                    nc.scalar.dma_start(xi_u[:], xi_v[u].rearrange(
                        "p (j t) -> p j t", j=2))
                    xi_issued.add(u)

                s_u = srp.tile([128, 2, TPC], F32, tag="s")
                nc.vector._custom_dve(SQSUM, out=s_u[:], in0=xr_tiles[u][:],
                                      in1=xi_u[:])
                # q overwrites w in place (stream op: reads lead writes)
                nc.vector._custom_dve(QABS, out=w_tiles[u][:], in0=xi_u[:],
                                      in1=w_tiles[u][:], s0=1e8)
                q_tiles[u] = w_tiles[u]

                r_u = srp.tile([128, 2, TPC], F32, tag="r")
                sq_i = nc.scalar.activation(r_u[:], s_u[:], AF.Sqrt,
                                            bias=sqrt_bias[:])
                sqrt_insts.append(sq_i)

                for h in range(2):
                    hs = slice(h * HALF, (h + 1) * HALF)
                    for half32 in range(2):
                        es = slice(32 * half32, 32 * half32 + 32)
                        nc.tensor.matmul(
                            ps[h][32 * half32:32 * half32 + 32, :],
                            wa_sb[:, 2 * u, es], r_u[:, 0, hs],
                            start=(u == 0), stop=False,
                            tile_position=(0, 32 * half32))
                        nc.tensor.matmul(
                            ps[h][64 + 32 * half32:96 + 32 * half32, :],
                            wa_sb[:, 2 * u + 1, es], r_u[:, 1, hs],
                            start=(u == 0), stop=False,
                            tile_position=(0, 64 + 32 * half32))

            # ---- tail: arctan (ACT) + quadrant fix (DVE) + phase mm ----
            atan_insts = []
            for u in range(NPAIR):
                # tail reuses the s/r rings (sqrt stream is done with them)
                at_u = srp.tile([128, 2, TPC], F32, tag="r")
                at_i = nc.scalar.activation(at_u[:], q_tiles[u][:], AF.Arctan)
                atan_insts.append(at_i)
                th_u = srp.tile([128, 2, TPC], F32, tag="s")
                nc.vector._custom_dve(THFIX, out=th_u[:], in0=at_u[:],
                                      in1=xr_tiles[u][:], s0=PI)
                last = (u == NPAIR - 1)
                for h in range(2):
                    hs = slice(h * HALF, (h + 1) * HALF)
                    for half32 in range(2):
                        es = slice(32 * half32, 32 * half32 + 32)
                        nc.tensor.matmul(
                            ps[h][32 * half32:32 * half32 + 32, :],
                            wp_sb[:, 2 * u, es], th_u[:, 0, hs],
                            start=False, stop=last,
                            tile_position=(0, 32 * half32))
                        nc.tensor.matmul(
                            ps[h][64 + 32 * half32:96 + 32 * half32, :],
                            wp_sb[:, 2 * u + 1, es], th_u[:, 1, hs],
                            start=False, stop=last,
                            tile_position=(0, 64 + 32 * half32))

            # ACT-stream table phase order: recip* < sqrt* < atan*
            from concourse.tile import add_dep_helper
            for a in sqrt_insts:
                for r_i in recip_insts:
                    add_dep_helper(a.ins, r_i.ins, sync=False,
                                   reason="ACT table phase order")
            for a in atan_insts:
                for s_i in sqrt_insts:
                    add_dep_helper(a.ins, s_i.ins, sync=False,
                                   reason="ACT table phase order")

            # ---- scores fixup: transpose S^T -> [tokens, E], top-2 ----
            # Stage-batched: per block transpose+copy+max8/find8 into shared
            # accumulators, then ONE sub/sub/sigmoid/cast over all 8 blocks.
            probs_acc = small.tile([128, NTT, 2], F32)
            idx_acc = small.tile([128, NTT, 2], I32)
            vals_all = small.tile([128, NTT, 8], F32)
            idxs_all = small.tile([128, NTT, 8], U32)
            d12_all = small.tile([128, NTT, 2], F32)
            for h in range(2):
                s_ev = small.tile([E, HALF], F32, tag="s_ev")
                nc.scalar.activation(s_ev[:], ps[h][0:64, :], AF.Identity,
                                     bias=b_sb[:])
                s_sb = small.tile([E, HALF], F32, tag="s_sb")
                nc.vector.tensor_tensor(out=s_sb[:], in0=s_ev[:],
                                        in1=ps[h][64:128, :], op=ALU.add)
                for c in range(HALF // 128):
                    n = h * (HALF // 128) + c
                    ps_t = ptrans.tile([128, E], F32, tag="pst")
                    nc.tensor.transpose(ps_t[:], s_sb[:, c * 128:(c + 1) * 128],
                                        id_sb[:])
                    sc_t = small.tile([128, E], F32, tag="sc_t", bufs=2)
                    nc.scalar.copy(sc_t[:], ps_t[:])
                    nc.vector.max_with_indices(vals_all[:, n, :],
                                               idxs_all[:, n, :], sc_t[:])
            nc.vector.tensor_sub(d12_all[:, :, 0:1], vals_all[:, :, 0:1],
                                 vals_all[:, :, 1:2])
            nc.vector.tensor_sub(d12_all[:, :, 1:2], vals_all[:, :, 1:2],
                                 vals_all[:, :, 0:1])
            nc.scalar.activation(probs_acc[:], d12_all[:], AF.Sigmoid)
            nc.vector.tensor_copy(idx_acc[:], idxs_all[:, :, 0:2])

            nc.sync.dma_start(
                o_probs.ap().rearrange("p (n k) -> p n k", k=2), probs_acc[:])
            nc.sync.dma_start(
                o_idx.ap().rearrange("p (n k) -> p n k", k=2), idx_acc[:])

    nc.compile()
    return nc


_NC_CACHE = None


def _get_nc():
    global _NC_CACHE
    if _NC_CACHE is None:
        _NC_CACHE = _build()
    return _NC_CACHE


def _make_in_maps(inputs):
    x_real = np.asarray(inputs["x_real"])
    x_imag = np.asarray(inputs["x_imag"])
    W = np.asarray(inputs["W"], dtype=np.float32)
    b = np.asarray(inputs["b"], dtype=np.float32)

    xr = x_real.reshape(TOKENS, D)
    xi = x_imag.reshape(TOKENS, D)

    # [128p, 16k, 64e] contiguous
    wa = np.ascontiguousarray(
        W[:D].reshape(KT, 128, E).transpose(1, 0, 2)).reshape(128, KT * E)
    wp = np.ascontiguousarray(
        W[D:].reshape(KT, 128, E).transpose(1, 0, 2)).reshape(128, KT * E)
    bvec = b.reshape(E, 1)
    ident = np.eye(E, dtype=np.float32)

    in_maps = []
    for c in range(N_CORES):
        sl = slice(c * TPC, (c + 1) * TPC)
        # [d, t] -> [u, p, j*1024 + t] with d = u*256 + j*128 + p
        xrT = np.ascontiguousarray(
            xr[sl].T.reshape(NPAIR, 2, 128, TPC).transpose(0, 2, 1, 3)
        ).reshape(NPAIR, 128, 2 * TPC)
        xiT = np.ascontiguousarray(
            xi[sl].T.reshape(NPAIR, 2, 128, TPC).transpose(0, 2, 1, 3)
        ).reshape(NPAIR, 128, 2 * TPC)
        in_maps.append({
            "xrU": xrT,
            "xiU": xiT,
            "wa": wa,
            "wp": wp,
            "bvec": bvec,
            "ident": ident,
        })
    return in_maps


def kernel(x_real, x_imag, W, b):
    NTT = TPC // 128
    in_maps = _make_in_maps(
        {"x_real": x_real, "x_imag": x_imag, "W": W, "b": b})
    nc = _get_nc()
    res = bass_utils.run_bass_kernel_spmd(nc, in_maps,
                                          core_ids=list(range(N_CORES)))

    probs = np.empty((TOKENS, TOPK), np.float32)
    idx = np.empty((TOKENS, TOPK), np.int32)
    for c in range(N_CORES):
        out = res.results[c]
        p = out["o_probs"].reshape(128, NTT, 2).transpose(1, 0, 2)
        i = out["o_idx"].reshape(128, NTT, 2).transpose(1, 0, 2)
        probs[c * TPC:(c + 1) * TPC] = p.reshape(TPC, 2)
        idx[c * TPC:(c + 1) * TPC] = i.reshape(TPC, 2)

    return (probs.reshape(B, S, TOPK), idx.reshape(B, S, TOPK))
